# revision 14
# baseline (speedup 1.0000x reference)
"""Depthwise 3x3 conv + sync BatchNorm (train mode) + ReLU6 on 8 Trainium2 cores.

Sharding: channels (192) split 24-per-core. Depthwise conv and BN are
per-channel independent, so no cross-core communication is needed.

Per-channel device pipeline (each core, 24 channels):
  - DMA in: zero-padded x slab [114, 32, 114] (H in partitions).
  - Conv as banded matmuls: for each W-tap dj, lhsT A_dj[k, m] = w[k-m, dj]
    (3-diagonal band). 8 image-groups of 4 accumulate 3 taps each in PSUM
    ([112, 448] = one bank per group).
  - ScalarE drains PSUM->SBUF with fused accum_out (per-partition sum).
  - DVE tensor_tensor_reduce computes per-partition sum of y^2.
  - Partition-collapse via ones-matmul -> scalar mean/var chain -> per-channel
    scale' = gamma*rsqrt(var+eps), bias' = beta - mean*scale' (conv bias b
    cancels exactly in train-mode BN, so it is never applied).
  - Broadcast (outer-product matmul) scale'/bias' to [112,1]; ScalarE applies
    Relu(scale'*y + bias'); gpsimd clamps to 6.0; DMA out.
"""

import numpy as np
from contextlib import ExitStack

import concourse.bass as bass
import concourse.mybir as mybir
import concourse.tile as tile
from concourse import bacc, bass_utils

FP32 = mybir.dt.float32
FP32R = mybir.dt.float32r  # tf32-style: full-rate PE streaming (fp32 is 1/4)
AF = mybir.ActivationFunctionType
ALU = mybir.AluOpType

N, C, H, W = 32, 192, 112, 112
NCORES = 8
CPC = C // NCORES          # 24 channels per core
HP, WP = H + 2, W + 2      # zero-padded spatial dims
G = 8                      # image groups (PSUM banks) per channel
IPG = N // G               # 4 images per group
NF = IPG * W               # 448 matmul free dim (fp32 PSUM bank limit 512)
NTOT = N * H * W           # BN reduction size per channel
BN_EPS = 1e-5


def _emit(ctx: ExitStack, tc, nc, x_d, a_d, gb_d, o_d, n_ch):
    a_pool = ctx.enter_context(tc.tile_pool(name="a", bufs=1))
    const_pool = ctx.enter_context(tc.tile_pool(name="const", bufs=1))
    x_pool = ctx.enter_context(tc.tile_pool(name="x", bufs=3))
    y_pool = ctx.enter_context(tc.tile_pool(name="y", bufs=4))
    z_pool = ctx.enter_context(tc.tile_pool(name="z", bufs=3))
    st_pool = ctx.enter_context(tc.tile_pool(name="st", bufs=2))
    sc_pool = ctx.enter_context(tc.tile_pool(name="sc", bufs=2))
    psum_y = ctx.enter_context(tc.tile_pool(name="py", bufs=5, space="PSUM"))
    psum_s = ctx.enter_context(tc.tile_pool(name="ps", bufs=2, space="PSUM"))
    psum_b = ctx.enter_context(tc.tile_pool(name="pb", bufs=1, space="PSUM"))

    a_all = a_pool.tile([HP, n_ch, 3, W], FP32R)
    nc.sync.dma_start(a_all[:], a_d.ap())
    gb = const_pool.tile([1, 2 * n_ch], FP32)
    nc.sync.dma_start(gb[:], gb_d.ap())
    ones_col = const_pool.tile([H, 1], FP32)   # lhsT for partition collapse
    nc.vector.memset(ones_col[:], 1.0)
    ones_row = const_pool.tile([1, H], FP32)   # lhsT for partition broadcast
    nc.vector.memset(ones_row[:], 1.0)
    eps_t = const_pool.tile([1, 1], FP32)      # BN eps as Sqrt bias operand
    nc.vector.memset(eps_t[:], BN_EPS)

    for c in range(n_ch):
        x_t = x_pool.tile([HP, N, WP], FP32R)
        nc.sync.dma_start(x_t[:], x_d.ap()[c])

        y_sb = y_pool.tile([H, G, NF], FP32)
        z_sb = z_pool.tile([H, G, IPG, W], FP32)
        bst = st_pool.tile([H, G, 6], FP32, tag="bst")

        for g in range(G):
            pt = psum_y.tile([H, NF], FP32, tag="pt")
            for dj in range(3):
                # float32r streams 1 col/cycle on the PE (fp32 is 4) at
                # sufficient moving size; accumulation in PSUM is fp32 anyway
                nc.tensor.matmul(
                    pt[:],
                    a_all[:, c, dj, :],
                    x_t[:, g * IPG:(g + 1) * IPG, dj:dj + W],
                    start=(dj == 0),
                    stop=(dj == 2),
                )
            nc.scalar.activation(y_sb[:, g, :], pt[:], AF.Copy, bias=0.0)
            nc.vector.bn_stats(bst[:, g, :], y_sb[:, g, :])

        # per-partition (mean, var) over this partition's 8*448 elements,
        # plus mean^2 in col 2 for the cross-partition variance combine
        # (tensor_scalar with AP scalars only: TENSOR_TENSOR is ~25x slower
        # on this HW for tiny operands)
        stats3 = st_pool.tile([H, 3], FP32, tag="stats3")
        nc.vector.bn_aggr(stats3[:, 0:2], bst[:])
        nc.vector.tensor_scalar(
            stats3[:, 2:3], stats3[:, 0:1], stats3[:, 0:1], None, op0=ALU.mult
        )

        # collapse partitions: [1, 3] = ones.T @ stats3 (equal counts per row)
        pst = psum_s.tile([1, 3], FP32, tag="pst")
        nc.tensor.matmul(pst[:], ones_col[:], stats3[:])

        # scalar chain (all on partition 0)
        em = sc_pool.tile([1, 3], FP32, tag="em")  # (mean, E_p[var], E_p[mean^2])
        nc.vector.tensor_scalar_mul(em[:], pst[:], 1.0 / H)
        m2 = sc_pool.tile([1, 1], FP32, tag="m2")
        nc.vector.tensor_scalar(m2[:], em[:, 0:1], em[:, 0:1], None, op0=ALU.mult)
        varr = sc_pool.tile([1, 1], FP32, tag="varr")  # batch var
        nc.vector.tensor_scalar(
            varr[:], em[:, 1:2], em[:, 2:3], m2[:], op0=ALU.add, op1=ALU.subtract
        )
        std = sc_pool.tile([1, 1], FP32, tag="std")    # sqrt(var + eps)
        nc.scalar.activation(std[:], varr[:], AF.Sqrt, bias=eps_t[:])
        istd = sc_pool.tile([1, 1], FP32, tag="istd")  # 1/sqrt(var+eps)
        nc.vector.reciprocal(istd[:], std[:])
        scpair = sc_pool.tile([1, 2], FP32, tag="scpair")
        nc.vector.tensor_scalar(
            scpair[:, 0:1], istd[:], gb[:, c:c + 1], None, op0=ALU.mult
        )
        msc = sc_pool.tile([1, 1], FP32, tag="msc")    # mean * scale'
        nc.vector.tensor_scalar(
            msc[:], em[:, 0:1], scpair[:, 0:1], None, op0=ALU.mult
        )
        nc.vector.tensor_scalar(
            scpair[:, 1:2], gb[:, n_ch + c:n_ch + c + 1], msc[:], None,
            op0=ALU.subtract,
        )

        # broadcast scale'/bias' across 112 partitions: outer product
        pb = psum_b.tile([H, 2], FP32, tag="pb")
        nc.tensor.matmul(pb[:], ones_row[:], scpair[:])
        bc = sc_pool.tile([H, 2], FP32, tag="bc")
        nc.vector.tensor_copy(bc[:], pb[:])

        # two half-channel normalize+clip+store pipelines for a shorter tail
        hg = G // 2
        for h2 in range(2):
            zf = z_sb[:, h2 * hg:(h2 + 1) * hg].rearrange("p g i w -> p (g i w)")
            nc.scalar.activation(
                zf,
                y_sb[:, h2 * hg:(h2 + 1) * hg, :].rearrange("p g f -> p (g f)"),
                AF.Relu, bias=bc[:, 1:2], scale=bc[:, 0:1],
            )
            nc.vector.tensor_scalar_min(zf, zf, 6.0)
            nc.sync.dma_start(
                o_d.ap()[c].rearrange("h (s n) w -> h s n w", s=2)[:, h2],
                z_sb[:, h2 * hg:(h2 + 1) * hg],
            )


def build_program(n_ch=CPC, enable_asserts=False):
    nc = bacc.Bacc(
        "TRN2",
        debug=False,
        enable_asserts=enable_asserts,
        target_bir_lowering=False,
        num_devices=NCORES,
    )
    x_d = nc.dram_tensor("x", (n_ch, HP, N, WP), FP32R, kind="ExternalInput")
    a_d = nc.dram_tensor("a", (HP, n_ch, 3, W), FP32R, kind="ExternalInput")
    gb_d = nc.dram_tensor("gb", (1, 2 * n_ch), FP32, kind="ExternalInput")
    o_d = nc.dram_tensor("o", (n_ch, H, N, W), FP32, kind="ExternalOutput")
    with tile.TileContext(nc) as tc:
        with ExitStack() as ctx:
            _emit(ctx, tc, nc, x_d, a_d, gb_d, o_d, n_ch)
    nc.compile()
    return nc


def make_core_inputs(inputs, w, gamma, beta, k, n_ch=CPC):
    """Host-side shard prep for core k: padded x slab, banded A matrices, gamma/beta."""
    ch = slice(k * n_ch, (k + 1) * n_ch)
    xk = np.zeros((n_ch, HP, N, WP), np.float32)
    xk[:, 1:1 + H, :, 1:1 + W] = np.asarray(inputs[:, ch]).transpose(1, 2, 0, 3)
    wk = np.asarray(w[ch]).astype(np.float32)          # (n_ch, 1, 3, 3)
    ak = np.zeros((n_ch, 3, HP, W), np.float32)
    m = np.arange(W)
    for di in range(3):
        # A[c, dj, m+di, m] = w[c, 0, di, dj]
        ak[:, :, m + di, m] = wk[:, 0, di, :][:, :, None]
    ak = np.ascontiguousarray(ak.transpose(2, 0, 1, 3))  # (HP, n_ch, 3, W)
    gbk = np.concatenate(
        [np.asarray(gamma[ch]), np.asarray(beta[ch])]
    ).astype(np.float32).reshape(1, 2 * n_ch)
    return {"x": xk, "a": ak, "gb": gbk}


_PROGRAM = None


def kernel(inputs, w, b, gamma, beta):
    global _PROGRAM
    if _PROGRAM is None:
        _PROGRAM = build_program()
    inputs = np.asarray(inputs, np.float32)
    in_maps = [make_core_inputs(inputs, w, gamma, beta, k) for k in range(NCORES)]
    res = bass_utils.run_bass_kernel_spmd(_PROGRAM, in_maps, list(range(NCORES)))
    out = np.empty((N, C, H, W), np.float32)
    for k in range(NCORES):
        # per-core output is (CPC, H, N, W)
        out[:, k * CPC:(k + 1) * CPC] = res.results[k]["o"].transpose(2, 0, 1, 3)
    return out


# revision 15
# speedup vs baseline: 1.0652x; 1.0652x over previous
"""Depthwise 3x3 conv + sync BatchNorm (train mode) + ReLU6 on 8 Trainium2 cores.

Sharding: channels (192) split 24-per-core. Depthwise conv and BN are
per-channel independent, so no cross-core communication is needed.

Per-channel device pipeline (each core, 24 channels):
  - DMA in: zero-padded x slab [114, 32, 114] (H in partitions).
  - Conv as banded matmuls: for each W-tap dj, lhsT A_dj[k, m] = w[k-m, dj]
    (3-diagonal band). 8 image-groups of 4 accumulate 3 taps each in PSUM
    ([112, 448] = one bank per group).
  - ScalarE drains PSUM->SBUF with fused accum_out (per-partition sum).
  - DVE tensor_tensor_reduce computes per-partition sum of y^2.
  - Partition-collapse via ones-matmul -> scalar mean/var chain -> per-channel
    scale' = gamma*rsqrt(var+eps), bias' = beta - mean*scale' (conv bias b
    cancels exactly in train-mode BN, so it is never applied).
  - Broadcast (outer-product matmul) scale'/bias' to [112,1]; ScalarE applies
    Relu(scale'*y + bias'); gpsimd clamps to 6.0; DMA out.
"""

import numpy as np
from contextlib import ExitStack

import concourse.bass as bass
import concourse.mybir as mybir
import concourse.tile as tile
from concourse import bacc, bass_utils

FP32 = mybir.dt.float32
FP32R = mybir.dt.float32r  # tf32-style: full-rate PE streaming (fp32 is 1/4)
AF = mybir.ActivationFunctionType
ALU = mybir.AluOpType

N, C, H, W = 32, 192, 112, 112
NCORES = 8
CPC = C // NCORES          # 24 channels per core
HP, WP = H + 2, W + 2      # zero-padded spatial dims
G = 8                      # image groups (PSUM banks) per channel
IPG = N // G               # 4 images per group
NF = IPG * W               # 448 matmul free dim (fp32 PSUM bank limit 512)
NTOT = N * H * W           # BN reduction size per channel
BN_EPS = 1e-5


def _emit(ctx: ExitStack, tc, nc, x_d, a_d, gb_d, o_d, n_ch):
    a_pool = ctx.enter_context(tc.tile_pool(name="a", bufs=1))
    const_pool = ctx.enter_context(tc.tile_pool(name="const", bufs=1))
    x_pool = ctx.enter_context(tc.tile_pool(name="x", bufs=3))
    y_pool = ctx.enter_context(tc.tile_pool(name="y", bufs=4))
    z_pool = ctx.enter_context(tc.tile_pool(name="z", bufs=3))
    st_pool = ctx.enter_context(tc.tile_pool(name="st", bufs=3))
    sc_pool = ctx.enter_context(tc.tile_pool(name="sc", bufs=3))
    st = {c: {} for c in range(n_ch)}
    psum_y = ctx.enter_context(tc.tile_pool(name="py", bufs=5, space="PSUM"))
    psum_s = ctx.enter_context(tc.tile_pool(name="ps", bufs=2, space="PSUM"))
    psum_b = ctx.enter_context(tc.tile_pool(name="pb", bufs=1, space="PSUM"))

    a_all = a_pool.tile([HP, n_ch, 3, W], FP32R)
    nc.sync.dma_start(a_all[:], a_d.ap())
    gb = const_pool.tile([1, 2 * n_ch], FP32)
    nc.sync.dma_start(gb[:], gb_d.ap())
    ones_col = const_pool.tile([H, 1], FP32)   # lhsT for partition collapse
    nc.vector.memset(ones_col[:], 1.0)
    ones_row = const_pool.tile([1, H], FP32)   # lhsT for partition broadcast
    nc.vector.memset(ones_row[:], 1.0)
    eps_t = const_pool.tile([1, 1], FP32)      # BN eps as Sqrt bias operand
    nc.vector.memset(eps_t[:], BN_EPS)

    def emit_conv(c):
        x_t = x_pool.tile([HP, N, WP], FP32R)
        nc.sync.dma_start(x_t[:], x_d.ap()[c])
        y_sb = y_pool.tile([H, G, NF], FP32)
        bst = st_pool.tile([H, G, 6], FP32, tag="bst")
        for g in range(G):
            pt = psum_y.tile([H, NF], FP32, tag="pt")
            for dj in range(3):
                nc.tensor.matmul(
                    pt[:],
                    a_all[:, c, dj, :],
                    x_t[:, g * IPG:(g + 1) * IPG, dj:dj + W],
                    start=(dj == 0),
                    stop=(dj == 2),
                )
            nc.scalar.activation(y_sb[:, g, :], pt[:], AF.Copy, bias=0.0)
            nc.vector.bn_stats(bst[:, g, :], y_sb[:, g, :])
        stats3 = st_pool.tile([H, 3], FP32, tag="stats3")
        nc.vector.bn_aggr(stats3[:, 0:2], bst[:])
        nc.vector.tensor_scalar(
            stats3[:, 2:3], stats3[:, 0:1], stats3[:, 0:1], None, op0=ALU.mult
        )
        st[c].update(y=y_sb, stats3=stats3)

    def emit_fin1(c):
        # partition collapse + per-channel scalar chain -> scpair
        stats3 = st[c]["stats3"]
        pst = psum_s.tile([1, 3], FP32, tag="pst")
        nc.tensor.matmul(pst[:], ones_col[:], stats3[:])
        em = sc_pool.tile([1, 3], FP32, tag="em")
        nc.vector.tensor_scalar_mul(em[:], pst[:], 1.0 / H)
        m2 = sc_pool.tile([1, 1], FP32, tag="m2")
        nc.vector.tensor_scalar(m2[:], em[:, 0:1], em[:, 0:1], None, op0=ALU.mult)
        varr = sc_pool.tile([1, 1], FP32, tag="varr")
        nc.vector.tensor_scalar(
            varr[:], em[:, 1:2], em[:, 2:3], m2[:], op0=ALU.add, op1=ALU.subtract
        )
        std = sc_pool.tile([1, 1], FP32, tag="std")
        nc.scalar.activation(std[:], varr[:], AF.Sqrt, bias=eps_t[:])
        istd = sc_pool.tile([1, 1], FP32, tag="istd")
        nc.vector.reciprocal(istd[:], std[:])
        scpair = sc_pool.tile([1, 2], FP32, tag="scpair")
        nc.vector.tensor_scalar(
            scpair[:, 0:1], istd[:], gb[:, c:c + 1], None, op0=ALU.mult
        )
        msc = sc_pool.tile([1, 1], FP32, tag="msc")
        nc.vector.tensor_scalar(
            msc[:], em[:, 0:1], scpair[:, 0:1], None, op0=ALU.mult
        )
        nc.vector.tensor_scalar(
            scpair[:, 1:2], gb[:, n_ch + c:n_ch + c + 1], msc[:], None,
            op0=ALU.subtract,
        )
        st[c]["scpair"] = scpair

    def emit_fin2(c):
        # broadcast scale'/bias' across partitions (outer product)
        pb = psum_b.tile([H, 2], FP32, tag="pb")
        nc.tensor.matmul(pb[:], ones_row[:], st[c]["scpair"][:])
        bc = sc_pool.tile([H, 2], FP32, tag="bc")
        nc.vector.tensor_copy(bc[:], pb[:])
        st[c]["bc"] = bc

    def emit_out(c):
        y_sb, bc = st[c]["y"], st[c]["bc"]
        z_sb = z_pool.tile([H, G, IPG, W], FP32, tag="z")
        hg = G // 2
        for h2 in range(2):
            zf = z_sb[:, h2 * hg:(h2 + 1) * hg].rearrange("p g i w -> p (g i w)")
            nc.scalar.activation(
                zf,
                y_sb[:, h2 * hg:(h2 + 1) * hg, :].rearrange("p g f -> p (g f)"),
                AF.Relu, bias=bc[:, 1:2], scale=bc[:, 0:1],
            )
            nc.vector.tensor_scalar_min(zf, zf, 6.0)
            nc.sync.dma_start(
                o_d.ap()[c].rearrange("h (s n) w -> h s n w", s=2)[:, h2],
                z_sb[:, h2 * hg:(h2 + 1) * hg],
            )

    # software pipeline: PE stream is [fin1(c-1) mm, fin2(c-2) mm, conv(c) mms]
    # so every PE instruction is dep-ready when reached (no in-order stalls)
    for c in range(n_ch):
        if c >= 1:
            emit_fin1(c - 1)
        if c >= 2:
            emit_fin2(c - 2)
        emit_conv(c)
        if c >= 2:
            emit_out(c - 2)
    emit_fin1(n_ch - 1)
    emit_fin2(n_ch - 2)
    emit_out(n_ch - 2)
    emit_fin2(n_ch - 1)
    emit_out(n_ch - 1)


def build_program(n_ch=CPC, enable_asserts=False):
    nc = bacc.Bacc(
        "TRN2",
        debug=False,
        enable_asserts=enable_asserts,
        target_bir_lowering=False,
        num_devices=NCORES,
    )
    x_d = nc.dram_tensor("x", (n_ch, HP, N, WP), FP32R, kind="ExternalInput")
    a_d = nc.dram_tensor("a", (HP, n_ch, 3, W), FP32R, kind="ExternalInput")
    gb_d = nc.dram_tensor("gb", (1, 2 * n_ch), FP32, kind="ExternalInput")
    o_d = nc.dram_tensor("o", (n_ch, H, N, W), FP32, kind="ExternalOutput")
    with tile.TileContext(nc) as tc:
        with ExitStack() as ctx:
            _emit(ctx, tc, nc, x_d, a_d, gb_d, o_d, n_ch)
    nc.compile()
    return nc


def make_core_inputs(inputs, w, gamma, beta, k, n_ch=CPC):
    """Host-side shard prep for core k: padded x slab, banded A matrices, gamma/beta."""
    ch = slice(k * n_ch, (k + 1) * n_ch)
    xk = np.zeros((n_ch, HP, N, WP), np.float32)
    xk[:, 1:1 + H, :, 1:1 + W] = np.asarray(inputs[:, ch]).transpose(1, 2, 0, 3)
    wk = np.asarray(w[ch]).astype(np.float32)          # (n_ch, 1, 3, 3)
    ak = np.zeros((n_ch, 3, HP, W), np.float32)
    m = np.arange(W)
    for di in range(3):
        # A[c, dj, m+di, m] = w[c, 0, di, dj]
        ak[:, :, m + di, m] = wk[:, 0, di, :][:, :, None]
    ak = np.ascontiguousarray(ak.transpose(2, 0, 1, 3))  # (HP, n_ch, 3, W)
    gbk = np.concatenate(
        [np.asarray(gamma[ch]), np.asarray(beta[ch])]
    ).astype(np.float32).reshape(1, 2 * n_ch)
    return {"x": xk, "a": ak, "gb": gbk}


_PROGRAM = None


def kernel(inputs, w, b, gamma, beta):
    global _PROGRAM
    if _PROGRAM is None:
        _PROGRAM = build_program()
    inputs = np.asarray(inputs, np.float32)
    in_maps = [make_core_inputs(inputs, w, gamma, beta, k) for k in range(NCORES)]
    res = bass_utils.run_bass_kernel_spmd(_PROGRAM, in_maps, list(range(NCORES)))
    out = np.empty((N, C, H, W), np.float32)
    for k in range(NCORES):
        # per-core output is (CPC, H, N, W)
        out[:, k * CPC:(k + 1) * CPC] = res.results[k]["o"].transpose(2, 0, 1, 3)
    return out


# revision 16
# speedup vs baseline: 1.0702x; 1.0047x over previous
"""Depthwise 3x3 conv + sync BatchNorm (train mode) + ReLU6 on 8 Trainium2 cores.

Sharding: channels (192) split 24-per-core. Depthwise conv and BN are
per-channel independent, so no cross-core communication is needed.

Per-channel device pipeline (each core, 24 channels):
  - DMA in: zero-padded x slab [114, 32, 114] (H in partitions).
  - Conv as banded matmuls: for each W-tap dj, lhsT A_dj[k, m] = w[k-m, dj]
    (3-diagonal band). 8 image-groups of 4 accumulate 3 taps each in PSUM
    ([112, 448] = one bank per group).
  - ScalarE drains PSUM->SBUF with fused accum_out (per-partition sum).
  - DVE tensor_tensor_reduce computes per-partition sum of y^2.
  - Partition-collapse via ones-matmul -> scalar mean/var chain -> per-channel
    scale' = gamma*rsqrt(var+eps), bias' = beta - mean*scale' (conv bias b
    cancels exactly in train-mode BN, so it is never applied).
  - Broadcast (outer-product matmul) scale'/bias' to [112,1]; ScalarE applies
    Relu(scale'*y + bias'); gpsimd clamps to 6.0; DMA out.
"""

import numpy as np
from contextlib import ExitStack

import concourse.bass as bass
import concourse.mybir as mybir
import concourse.tile as tile
from concourse import bacc, bass_utils

FP32 = mybir.dt.float32
FP32R = mybir.dt.float32r  # tf32-style: full-rate PE streaming (fp32 is 1/4)
AF = mybir.ActivationFunctionType
ALU = mybir.AluOpType

N, C, H, W = 32, 192, 112, 112
NCORES = 8
CPC = C // NCORES          # 24 channels per core
HP, WP = H + 2, W + 2      # zero-padded spatial dims
G = 8                      # image groups (PSUM banks) per channel
IPG = N // G               # 4 images per group
NF = IPG * W               # 448 matmul free dim (fp32 PSUM bank limit 512)
NTOT = N * H * W           # BN reduction size per channel
BN_EPS = 1e-5


def _emit(ctx: ExitStack, tc, nc, x_d, a_d, gb_d, o_d, n_ch):
    a_pool = ctx.enter_context(tc.tile_pool(name="a", bufs=1))
    const_pool = ctx.enter_context(tc.tile_pool(name="const", bufs=1))
    x_pool = ctx.enter_context(tc.tile_pool(name="x", bufs=3))
    y_pool = ctx.enter_context(tc.tile_pool(name="y", bufs=4))
    z_pool = ctx.enter_context(tc.tile_pool(name="z", bufs=3))
    st_pool = ctx.enter_context(tc.tile_pool(name="st", bufs=3))
    sc_pool = ctx.enter_context(tc.tile_pool(name="sc", bufs=3))
    st = {c: {} for c in range(n_ch)}
    psum_y = ctx.enter_context(tc.tile_pool(name="py", bufs=5, space="PSUM"))
    psum_s = ctx.enter_context(tc.tile_pool(name="ps", bufs=2, space="PSUM"))
    psum_b = ctx.enter_context(tc.tile_pool(name="pb", bufs=1, space="PSUM"))

    a_all = a_pool.tile([HP, n_ch, 3, W], FP32R)
    nc.sync.dma_start(a_all[:], a_d.ap())
    gb = const_pool.tile([1, 2 * n_ch], FP32)
    nc.sync.dma_start(gb[:], gb_d.ap())
    ones_col = const_pool.tile([H, 1], FP32)   # lhsT for partition collapse
    nc.vector.memset(ones_col[:], 1.0)
    ones_row = const_pool.tile([1, H], FP32)   # lhsT for partition broadcast
    nc.vector.memset(ones_row[:], 1.0)
    eps_t = const_pool.tile([1, 1], FP32)      # BN eps as Sqrt bias operand
    nc.vector.memset(eps_t[:], BN_EPS)

    def emit_conv(c):
        x_t = x_pool.tile([HP, N, WP], FP32R)
        nc.sync.dma_start(x_t[:], x_d.ap()[c])
        y_sb = y_pool.tile([H, G, NF], FP32)
        bst = st_pool.tile([H, G, 6], FP32, tag="bst")
        for g in range(G):
            pt = psum_y.tile([H, NF], FP32, tag="pt")
            for dj in range(3):
                nc.tensor.matmul(
                    pt[:],
                    a_all[:, c, dj, :],
                    x_t[:, g * IPG:(g + 1) * IPG, dj:dj + W],
                    start=(dj == 0),
                    stop=(dj == 2),
                )
            nc.scalar.activation(y_sb[:, g, :], pt[:], AF.Copy, bias=0.0)
            nc.vector.bn_stats(bst[:, g, :], y_sb[:, g, :])
        stats3 = st_pool.tile([H, 3], FP32, tag="stats3")
        nc.vector.bn_aggr(stats3[:, 0:2], bst[:])
        nc.vector.tensor_scalar(
            stats3[:, 2:3], stats3[:, 0:1], stats3[:, 0:1], None, op0=ALU.mult
        )
        st[c].update(y=y_sb, stats3=stats3)

    def emit_fin1(c):
        # partition collapse + per-channel scalar chain -> scpair
        stats3 = st[c]["stats3"]
        pst = psum_s.tile([1, 3], FP32, tag="pst")
        nc.tensor.matmul(pst[:], ones_col[:], stats3[:])
        em = sc_pool.tile([1, 3], FP32, tag="em")
        nc.vector.tensor_scalar_mul(em[:], pst[:], 1.0 / H)
        m2 = sc_pool.tile([1, 1], FP32, tag="m2")
        nc.vector.tensor_scalar(m2[:], em[:, 0:1], em[:, 0:1], None, op0=ALU.mult)
        varr = sc_pool.tile([1, 1], FP32, tag="varr")
        nc.vector.tensor_scalar(
            varr[:], em[:, 1:2], em[:, 2:3], m2[:], op0=ALU.add, op1=ALU.subtract
        )
        std = sc_pool.tile([1, 1], FP32, tag="std")
        nc.scalar.activation(std[:], varr[:], AF.Sqrt, bias=eps_t[:])
        istd = sc_pool.tile([1, 1], FP32, tag="istd")
        nc.vector.reciprocal(istd[:], std[:])
        scpair = sc_pool.tile([1, 2], FP32, tag="scpair")
        nc.vector.tensor_scalar(
            scpair[:, 0:1], istd[:], gb[:, c:c + 1], None, op0=ALU.mult
        )
        msc = sc_pool.tile([1, 1], FP32, tag="msc")
        nc.vector.tensor_scalar(
            msc[:], em[:, 0:1], scpair[:, 0:1], None, op0=ALU.mult
        )
        nc.vector.tensor_scalar(
            scpair[:, 1:2], gb[:, n_ch + c:n_ch + c + 1], msc[:], None,
            op0=ALU.subtract,
        )
        st[c]["scpair"] = scpair

    def emit_fin2(c):
        # broadcast scale'/bias' across partitions (outer product)
        pb = psum_b.tile([H, 2], FP32, tag="pb")
        nc.tensor.matmul(pb[:], ones_row[:], st[c]["scpair"][:])
        bc = sc_pool.tile([H, 2], FP32, tag="bc")
        nc.vector.tensor_copy(bc[:], pb[:])
        st[c]["bc"] = bc

    def emit_out(c):
        y_sb, bc = st[c]["y"], st[c]["bc"]
        z_sb = z_pool.tile([H, G, IPG, W], FP32, tag="z")
        hg = G // 2
        for h2 in range(2):
            zf = z_sb[:, h2 * hg:(h2 + 1) * hg].rearrange("p g i w -> p (g i w)")
            nc.scalar.activation(
                zf,
                y_sb[:, h2 * hg:(h2 + 1) * hg, :].rearrange("p g f -> p (g f)"),
                AF.Relu, bias=bc[:, 1:2], scale=bc[:, 0:1],
            )
            nc.vector.tensor_scalar_min(zf, zf, 6.0)
            # SWDGE ring: keeps the in-order Sync ring free for x prefetches
            nc.gpsimd.dma_start(
                o_d.ap()[c].rearrange("h (s n) w -> h s n w", s=2)[:, h2],
                z_sb[:, h2 * hg:(h2 + 1) * hg],
            )

    # software pipeline: PE stream is [fin1(c-1) mm, fin2(c-2) mm, conv(c) mms]
    # so every PE instruction is dep-ready when reached (no in-order stalls)
    for c in range(n_ch):
        if c >= 1:
            emit_fin1(c - 1)
        if c >= 2:
            emit_fin2(c - 2)
        emit_conv(c)
        if c >= 2:
            emit_out(c - 2)
    emit_fin1(n_ch - 1)
    emit_fin2(n_ch - 2)
    emit_out(n_ch - 2)
    emit_fin2(n_ch - 1)
    emit_out(n_ch - 1)


def build_program(n_ch=CPC, enable_asserts=False):
    nc = bacc.Bacc(
        "TRN2",
        debug=False,
        enable_asserts=enable_asserts,
        target_bir_lowering=False,
        num_devices=NCORES,
    )
    x_d = nc.dram_tensor("x", (n_ch, HP, N, WP), FP32R, kind="ExternalInput")
    a_d = nc.dram_tensor("a", (HP, n_ch, 3, W), FP32R, kind="ExternalInput")
    gb_d = nc.dram_tensor("gb", (1, 2 * n_ch), FP32, kind="ExternalInput")
    o_d = nc.dram_tensor("o", (n_ch, H, N, W), FP32, kind="ExternalOutput")
    with tile.TileContext(nc) as tc:
        with ExitStack() as ctx:
            _emit(ctx, tc, nc, x_d, a_d, gb_d, o_d, n_ch)
    nc.compile()
    return nc


def make_core_inputs(inputs, w, gamma, beta, k, n_ch=CPC):
    """Host-side shard prep for core k: padded x slab, banded A matrices, gamma/beta."""
    ch = slice(k * n_ch, (k + 1) * n_ch)
    xk = np.zeros((n_ch, HP, N, WP), np.float32)
    xk[:, 1:1 + H, :, 1:1 + W] = np.asarray(inputs[:, ch]).transpose(1, 2, 0, 3)
    wk = np.asarray(w[ch]).astype(np.float32)          # (n_ch, 1, 3, 3)
    ak = np.zeros((n_ch, 3, HP, W), np.float32)
    m = np.arange(W)
    for di in range(3):
        # A[c, dj, m+di, m] = w[c, 0, di, dj]
        ak[:, :, m + di, m] = wk[:, 0, di, :][:, :, None]
    ak = np.ascontiguousarray(ak.transpose(2, 0, 1, 3))  # (HP, n_ch, 3, W)
    gbk = np.concatenate(
        [np.asarray(gamma[ch]), np.asarray(beta[ch])]
    ).astype(np.float32).reshape(1, 2 * n_ch)
    return {"x": xk, "a": ak, "gb": gbk}


_PROGRAM = None


def kernel(inputs, w, b, gamma, beta):
    global _PROGRAM
    if _PROGRAM is None:
        _PROGRAM = build_program()
    inputs = np.asarray(inputs, np.float32)
    in_maps = [make_core_inputs(inputs, w, gamma, beta, k) for k in range(NCORES)]
    res = bass_utils.run_bass_kernel_spmd(_PROGRAM, in_maps, list(range(NCORES)))
    out = np.empty((N, C, H, W), np.float32)
    for k in range(NCORES):
        # per-core output is (CPC, H, N, W)
        out[:, k * CPC:(k + 1) * CPC] = res.results[k]["o"].transpose(2, 0, 1, 3)
    return out
